# revision 31
# baseline (speedup 1.0000x reference)
"""Multi-Latent Attention TRN2 kernel, v3: absorbed weights + hybrid sharding,
chunk/attention interleaving, batched exp, divide-based softmax normalize.

Sharding: 2-way data parallel on batch x 4-way tensor parallel on heads.
Core c handles batch b = c // 4 and heads hg*4..hg*4+3 where hg = c % 4.
Each core computes a partial [S, D] output for its batch (contracting only
its heads' latent features); the host sums 4 partials per batch and adds
the folded output bias.

Weight absorption (exact algebra, done host-side in fp32):
  scores: s = (x Wq_h + bq_h) . (latk_h Wkr + bkr)
    per-row constants are softmax-invariant -> bkr, blk terms dropped;
    q~ = x (Wq_h Wkr^T) + bq_h Wkr^T; latk0 = x Wlk_h; contraction L=64.
  values: attn @ v_h = (attn latv0_h) @ Wvr + const row
    -> Wo_eff_h = Wvr @ Wo_h folded host-side; const row into bo_eff.

Pipeline per chunk c (512 tokens): project q~/latk/latv for chunk c,
emit out-projection for q-block c-1, then attention for q-block c.
Keeps PE dense (no HAM re-throttle) and hides ScalarE exp latency.

Scores for the two heads of a pair are K=64 matmuls running concurrently
on PE row-tiles (0,0)/(64,0). Score tiles go to [128,1024] 2-bank PSUM
groups so one ACTIVATE exps 1024 columns (amortizes the 352-cycle ramp).
Diagonal tiles compute full 512 wide; above-diagonal prefixes are zeroed
by GpSimd memsets, the 128x128 triangle by a mask multiply.

U~ chain lhsT is latv with an appended ones column: out row 64 = softmax
denominator for free. Normalize = replicate row 64 across partitions via
two K=1 M=64 matmuls into one PSUM bank, cast, then tensor_tensor divide.
"""

import math
from contextlib import ExitStack

import numpy as np

import concourse.mybir as mybir
from concourse import bacc
from concourse.bass import ds, ts
from concourse.tile import TileContext

# Problem constants (hardcoded per contract).
B, S, D = 2, 2048, 2048
H, DK, DV, L = 16, 128, 128, 64
N_CORES = 8
HPC = 4                   # heads per core
NPAIR = 2                 # head pairs per core
SB = S                    # tokens per core (its batch)
KO = D // 128             # contraction k-tiles over D = 16
CHUNK = 512               # token chunk for input streaming
NCH = SB // CHUNK         # 4
QT = SB // 128            # 16 token tiles
LW = L + 1                # latv group width (64 latents + ones col)

F32 = mybir.dt.float32
F32R = mybir.dt.float32r
BF16 = mybir.dt.bfloat16

INV_SQRT_DK = 1.0 / math.sqrt(DK)
EXPF = mybir.ActivationFunctionType.Exp


def build_kernel():
    nc = bacc.Bacc(trn_type="TRN2", debug=False, num_swdge_queues=2)

    # ---- DRAM I/O (all host-packed for contiguous DMA) ----
    xq = nc.dram_tensor("xq", [NCH, 128, KO, CHUNK], BF16, kind="ExternalInput")
    xk = nc.dram_tensor("xk", [NCH, 128, KO, CHUNK], BF16, kind="ExternalInput")
    xv = nc.dram_tensor("xv", [NCH, 128, KO, CHUNK], BF16, kind="ExternalInput")
    wq = nc.dram_tensor("wq", [NPAIR, 128, KO, 128], BF16, kind="ExternalInput")
    bq = nc.dram_tensor("bq", [128, NPAIR], F32, kind="ExternalInput")
    wlk = nc.dram_tensor("wlk", [128, KO, 128 * NPAIR], BF16, kind="ExternalInput")
    wlv = nc.dram_tensor("wlv", [128, KO, HPC * L], BF16, kind="ExternalInput")
    wo = nc.dram_tensor("wo", [128, NPAIR, D], BF16, kind="ExternalInput")
    outp = nc.dram_tensor("outp", [QT, 128, D], BF16, kind="ExternalOutput")

    with TileContext(nc) as tc, ExitStack() as ctx:
        ec = ctx.enter_context
        consts = ec(tc.tile_pool(name="consts", bufs=1))
        persist = ec(tc.tile_pool(name="persist", bufs=1))
        xpool = ec(tc.tile_pool(name="xpool", bufs=3))
        ptpool = ec(tc.tile_pool(name="ptpool", bufs=2))
        statpool = ec(tc.tile_pool(name="statpool", bufs=2))
        opool = ec(tc.tile_pool(name="opool", bufs=2))
        psa = ec(tc.tile_pool(name="psa", bufs=2, space="PSUM"))
        pss = ec(tc.tile_pool(name="pss", bufs=2, space="PSUM"))
        psu = ec(tc.tile_pool(name="psu", bufs=2, space="PSUM"))

        # ---- first loads, split small on parallel HWDGE queues so the
        # ---- first q~ chain can start ~13us in instead of ~24us ----
        wq_sb = consts.tile([128, NPAIR, KO, 128], BF16, tag="wq")
        nc.sync.dma_start(wq_sb[:, 0], wq[0])
        x0 = []
        t = xpool.tile([128, KO, CHUNK], BF16, tag="x", name="xq_t")
        for i in range(4):
            nc.scalar.dma_start(t[:, 4 * i : 4 * i + 4, :],
                                xq[0][:, 4 * i : 4 * i + 4, :])
        x0.append(t)
        nc.sync.dma_start(wq_sb[:, 1], wq[1])
        wlk_sb = consts.tile([128, KO, 128 * NPAIR], BF16, tag="wlk")
        nc.sync.dma_start(wlk_sb, wlk[:, :, :])
        t = xpool.tile([128, KO, CHUNK], BF16, tag="x", name="xk_t")
        nc.scalar.dma_start(t[:, 0:8, :], xk[0][:, 0:8, :])
        nc.scalar.dma_start(t[:, 8:16, :], xk[0][:, 8:16, :])
        x0.append(t)
        wlv_sb = consts.tile([128, KO, HPC * L], BF16, tag="wlv")
        nc.sync.dma_start(wlv_sb, wlv[:, :, :])
        bq_sb = consts.tile([128, NPAIR], F32, tag="bq")
        nc.scalar.dma_start(bq_sb, bq[:, :])
        t = xpool.tile([128, KO, CHUNK], BF16, tag="x", name="xv_t")
        nc.sync.dma_start(t, xv[0])
        x0.append(t)
        wo_sb = consts.tile([128, NPAIR, D], BF16, tag="wo")
        nc.gpsimd.dma_start(wo_sb, wo[:, :, :])

        # causal mask for a diagonal 128x128 block of P~^T: 1 where k <= q
        maskT = consts.tile([128, 128], BF16, tag="maskT")
        nc.gpsimd.memset(maskT, 1.0)
        nc.gpsimd.affine_select(
            out=maskT, in_=maskT, compare_op=mybir.AluOpType.is_ge,
            fill=0.0, base=0, pattern=[[1, 128]], channel_multiplier=-1,
        )
        # all-ones tile: K=1 lhsT rows for denominator replication
        ones_r = consts.tile([128, 128], BF16, tag="ones_r")
        nc.gpsimd.memset(ones_r, 1.0)

        # ---- persistent per-batch tensors ----
        qsb = persist.tile([128, NPAIR, SB], BF16, tag="qsb")
        ksb = persist.tile([128, NPAIR, SB], BF16, tag="ksb")
        vsb = persist.tile([128, QT, HPC * LW], BF16, tag="vsb")
        asb = persist.tile([128, NPAIR, SB], BF16, tag="asb")
        for h in range(HPC):
            nc.gpsimd.memset(vsb[:, :, LW * h + L : LW * h + L + 1], 1.0)

        def norm_unit(Q, usb_q, rcp_q, p, r):
            qsl = ds(Q * 512, 512)
            a_sl = asb[:, p, qsl]
            h = 2 * p + r
            ps_rep = psa.tile([128, 512], F32, tag="s", name="ps_rep")
            nc.tensor.matmul(
                ps_rep, ones_r[32 * h : 32 * h + 1, :],
                rcp_q[32 * h : 32 * h + 1, :],
                start=True, stop=True,
                tile_position=(32 * h, 0),
            )
            dsb = statpool.tile([128, 512], F32, tag="dsb", name="dsb")
            nc.any.tensor_copy(out=dsb, in_=ps_rep)
            nc.vector.tensor_tensor(
                a_sl[64 * r : 64 * r + 64, :],
                usb_q[p][r][0:64, :],
                dsb[0:64, :],
                mybir.AluOpType.mult,
            )

        def out_proj_unit(Q, tl):
            tt = Q * 4 + tl
            o_sb = opool.tile([128, D], BF16, tag="o", name="o_sb")
            for dc2 in range(2):
                ps_f = [psa.tile([128, 512], F32, tag="s", name="ps_f")
                        for _ in range(2)]
                for kk in range(NPAIR):
                    for i in range(2):  # one LDWEIGHTS feeds two matmuls
                        nc.tensor.matmul(
                            ps_f[i], asb[:, kk, ts(tt, 128)],
                            wo_sb[:, kk, ts(2 * dc2 + i, 512)],
                            start=(kk == 0), stop=(kk == NPAIR - 1),
                        )
                for i in range(2):
                    nc.any.tensor_copy(out=o_sb[:, ts(2 * dc2 + i, 512)],
                                       in_=ps_f[i])
            nc.sync.dma_start(outp[tt], o_sb)

        for c in range(NCH):
            # ---- stream chunk c and project q~ / latk / latv ----
            if c == 0:
                xq_t, xk_t, xv_t = x0
            else:
                xq_t = xpool.tile([128, KO, CHUNK], BF16, tag="x")
                nc.sync.dma_start(xq_t, xq[c])
                xk_t = xpool.tile([128, KO, CHUNK], BF16, tag="x")
                nc.sync.dma_start(xk_t, xk[c])
                xv_t = xpool.tile([128, KO, CHUNK], BF16, tag="x")
                nc.scalar.dma_start(xv_t, xv[c])

            csl = ds(c * CHUNK, CHUNK)
            for m in range(NPAIR):
                ps = psa.tile([128, 512], F32, tag="s", name="ps_q")
                for ko in range(KO):
                    nc.tensor.matmul(
                        ps, wq_sb[:, m, ko, :], xq_t[:, ko, :],
                        start=(ko == 0), stop=(ko == KO - 1),
                    )
                nc.vector.tensor_scalar_add(
                    qsb[:, m, csl], ps, bq_sb[:, m : m + 1])
            for m in range(NPAIR):
                ps = psa.tile([128, 512], F32, tag="s", name="ps_k")
                for ko in range(KO):
                    nc.tensor.matmul(
                        ps, wlk_sb[:, ko, ts(m, 128)], xk_t[:, ko, :],
                        start=(ko == 0), stop=(ko == KO - 1),
                    )
                nc.any.tensor_copy(out=ksb[:, m, csl], in_=ps)
            # latv: token-major, [128 tok, 256] per token tile
            for tl in range(4):
                tt = c * 4 + tl
                ps = psa.tile([128, 512], F32, tag="s", name="ps_v")
                for ko in range(KO):
                    nc.tensor.matmul(
                        ps[:, : HPC * L], xv_t[:, ko, ts(tl, 128)],
                        wlv_sb[:, ko, :],
                        start=(ko == 0), stop=(ko == KO - 1),
                    )
                for h in range(HPC):
                    nc.any.tensor_copy(
                        out=vsb[:, tt, ds(LW * h, L)],
                        in_=ps[:, ds(L * h, L)],
                    )

            # ---- attention for q-block Q = c, woven with the previous
            # ---- q-block's normalize + out-projection as PE filler ----
            Q = c
            jmax = 4 * Q + 4
            qsl = ds(Q * 512, 512)

            filler = []
            if c > 0:
                pv, uq, rq = c - 1, usb_q, rcp_q
                for pp in range(NPAIR):
                    for rr in range(2):
                        filler.append(
                            (lambda pp=pp, rr=rr: norm_unit(pv, uq, rq,
                                                            pp, rr)))
                for tl in range(4):
                    filler.append(lambda tl=tl, pv=pv: out_proj_unit(pv, tl))

            usb_q = [None] * NPAIR
            den = statpool.tile([128, 512], F32, tag="den", name="den")
            rcp_q = statpool.tile([128, 512], BF16, tag="rcp", name="rcp")

            ngroups = NPAIR * (jmax // 2)
            gdone = 0
            fdone = 0
            for p in range(NPAIR):
                pt = [ptpool.tile([128, QT, 512], BF16, tag=f"pt{r}",
                                  name=f"pt{r}") for r in range(2)]
                ps_u = [psu.tile([65, 512], F32, tag="u", name="ps_u")
                        for _ in range(2)]

                def u_group(g):
                    for j2 in range(2):
                        j = 2 * g + j2
                        for r in range(2):
                            nc.tensor.matmul(
                                ps_u[r],
                                vsb[:, j, ds(LW * (2 * p + r), LW)],
                                pt[r][:, j, :],
                                start=(j == 0), stop=(j == jmax - 1),
                            )

                for g in range(jmax // 2):
                    # scores: r-adjacent so the two heads run on PE
                    # row-tiles (0,0)/(64,0) concurrently
                    sgs = [pss.tile([128, 1024], F32, tag="sg", name="sg")
                           for _ in range(2)]
                    for j2 in range(2):
                        j = 2 * g + j2
                        for r in range(2):
                            rs = slice(64 * r, 64 * r + 64)
                            nc.tensor.matmul(
                                sgs[r][:, ds(512 * j2, 512)],
                                ksb[rs, p, ts(j, 128)], qsb[rs, p, qsl],
                                start=True, stop=True,
                            )
                    for r in range(2):
                        nc.scalar.activation(
                            pt[r][:, 2 * g : 2 * g + 2, :], sgs[r],
                            EXPF, scale=INV_SQRT_DK,
                        )
                    # causal masking on diagonal k-tiles (GpSimd: idle)
                    for j2 in range(2):
                        j = 2 * g + j2
                        i = j - 4 * Q
                        if i < 0:
                            continue
                        for r in range(2):
                            if i > 0:
                                nc.gpsimd.memset(
                                    pt[r][:, j, ds(0, 128 * i)], 0.0)
                            nc.gpsimd.tensor_tensor(
                                pt[r][:, j, ds(128 * i, 128)],
                                pt[r][:, j, ds(128 * i, 128)],
                                maskT, mybir.AluOpType.mult,
                            )
                    # U~ for the previous group (exp had time to finish)
                    if g > 0:
                        u_group(g - 1)
                    gdone += 1
                    # weave in filler units proportionally
                    want = (gdone * len(filler)) // ngroups
                    while fdone < want:
                        filler[fdone]()
                        fdone += 1
                u_group(jmax // 2 - 1)

                # evacuate U~ to SBUF (frees PSUM); pack the q-block's 4
                # denominator rows at 32-aligned partitions of one tile
                usb_q[p] = [statpool.tile([128, 512], BF16,
                                          tag=f"usb{2 * p + r}", name="usb")
                            for r in range(2)]
                for r in range(2):
                    h = 2 * p + r
                    nc.any.tensor_copy(out=usb_q[p][r][0:64, :],
                                       in_=ps_u[r][0:64, :])
                    nc.any.tensor_copy(
                        out=den[32 * h : 32 * h + 1, :],
                        in_=ps_u[r][64:65, :])
                if c == NCH - 1 and p == 0:
                    with nc.allow_low_precision(reason="1/d in bf16, "
                                                "within tolerance"):
                        nc.vector.reciprocal(rcp_q[0:64, :], den[0:64, :])

            while fdone < len(filler):
                filler[fdone]()
                fdone += 1

            # one reciprocal covers all 4 heads of this q-block (split
            # per pair on the last block so the tail norm never waits)
            with nc.allow_low_precision(reason="1/d in bf16 adds ~0.4% "
                                        "scale noise, within tolerance"):
                if c == NCH - 1:
                    nc.vector.reciprocal(rcp_q[64:128, :], den[64:128, :])
                else:
                    nc.vector.reciprocal(rcp_q, den)

        for pp in range(NPAIR):
            for rr in range(2):
                norm_unit(NCH - 1, usb_q, rcp_q, pp, rr)
        for tl in range(4):
            out_proj_unit(NCH - 1, tl)

    nc.finalize()
    return nc


_NC_CACHE = None


def _get_nc():
    global _NC_CACHE
    if _NC_CACHE is None:
        _NC_CACHE = build_kernel()
    return _NC_CACHE


def _pack_xT(Xb, bf16):
    # Xb [S, D] fp32 -> X^T packed [NCH, 128, KO, CHUNK] (d = ko*128 + p)
    xt = np.asarray(Xb).T.reshape(KO, 128, NCH, CHUNK)
    return np.ascontiguousarray(xt.transpose(2, 1, 0, 3).astype(bf16))


def _prep_in_maps(queries, keys, values, Wq, bq, Wlk, blk, Wlv, blv,
                  Wkr, bkr, Wvr, bvr, Wo, bo):
    import ml_dtypes

    bf16 = ml_dtypes.bfloat16
    f = np.float32
    Wq, bq, Wlk, Wlv = (np.asarray(a, f) for a in (Wq, bq, Wlk, Wlv))
    Wkr, Wvr, Wo = (np.asarray(a, f) for a in (Wkr, Wvr, Wo))

    # host-side absorption folds (exact algebra)
    # Wq_eff_h = Wq_h @ Wkr^T [D, L]; bq_eff_h = bq_h @ Wkr^T
    WqH = Wq.reshape(D, H, DK)
    Wq_eff = np.einsum("dhk,lk->dhl", WqH, Wkr).reshape(D, H * L)
    bq_eff = (bq.reshape(H, DK) @ Wkr.T).reshape(H * L)
    # Wo_eff_h = Wvr @ Wo_h [L, D]
    WoH = Wo.reshape(H, DV, D)
    Wo_eff = np.einsum("lk,hkd->hld", Wvr, WoH).reshape(H * L, D)

    in_maps = []
    for c in range(N_CORES):
        b, hg = c // 4, c % 4
        hsl = slice(hg * 4 * L, (hg + 1) * 4 * L)     # 4 heads' latent cols

        xq_c = _pack_xT(queries[b], bf16)
        xk_c = _pack_xT(keys[b], bf16)
        xv_c = _pack_xT(values[b], bf16)

        wq_c = np.ascontiguousarray(
            Wq_eff[:, hsl].reshape(KO, 128, NPAIR, 128)
            .transpose(2, 1, 0, 3).astype(bf16))
        bq_c = np.ascontiguousarray(
            bq_eff[hsl].reshape(NPAIR, 128).T, f)
        wlk_c = np.ascontiguousarray(
            Wlk[:, hsl].reshape(KO, 128, 256).transpose(1, 0, 2).astype(bf16))
        wlv_c = np.ascontiguousarray(
            Wlv[:, hsl].reshape(KO, 128, 256).transpose(1, 0, 2).astype(bf16))
        wo_c = np.ascontiguousarray(
            Wo_eff[hsl, :].reshape(NPAIR, 128, D).transpose(1, 0, 2)
            .astype(bf16))

        in_maps.append({
            "xq": xq_c, "xk": xk_c, "xv": xv_c,
            "wq": wq_c, "bq": bq_c, "wlk": wlk_c, "wlv": wlv_c, "wo": wo_c,
        })
    return in_maps


def _assemble(results, inputs):
    f64 = np.float64
    blv = np.asarray(inputs["blv"], f64).reshape(H, L)
    bvr = np.asarray(inputs["bvr"], f64)
    Wvr = np.asarray(inputs["Wvr"], f64)
    WoH = np.asarray(inputs["Wo"], f64).reshape(H, DV, D)
    bo_eff = np.asarray(inputs["bo"], f64).copy()
    for h in range(H):
        bo_eff += (blv[h] @ Wvr + bvr) @ WoH[h]

    out = np.zeros((B, S, D), f64)
    for c, rmap in enumerate(results):
        out[c // 4] += rmap["outp"].reshape(S, D).astype(f64)
    out += bo_eff
    return out.astype(np.float32)


def kernel(**inputs):
    from concourse.bass_utils import run_bass_kernel_spmd

    nc = _get_nc()
    in_maps = _prep_in_maps(**inputs)
    res = run_bass_kernel_spmd(
        nc, in_maps, core_ids=list(range(N_CORES)), trace=False
    )
    return _assemble(res.results, inputs)


if __name__ == "__main__":
    nc = build_kernel()
    print("built ok, instructions:", len(nc.inst_map))


# revision 33
# speedup vs baseline: 1.0760x; 1.0760x over previous
"""Multi-Latent Attention TRN2 kernel, v3: absorbed weights + hybrid sharding,
chunk/attention interleaving, batched exp, divide-based softmax normalize.

Sharding: 2-way data parallel on batch x 4-way tensor parallel on heads.
Core c handles batch b = c // 4 and heads hg*4..hg*4+3 where hg = c % 4.
Each core computes a partial [S, D] output for its batch (contracting only
its heads' latent features); the host sums 4 partials per batch and adds
the folded output bias.

Weight absorption (exact algebra, done host-side in fp32):
  scores: s = (x Wq_h + bq_h) . (latk_h Wkr + bkr)
    per-row constants are softmax-invariant -> bkr, blk terms dropped;
    q~ = x (Wq_h Wkr^T) + bq_h Wkr^T; latk0 = x Wlk_h; contraction L=64.
  values: attn @ v_h = (attn latv0_h) @ Wvr + const row
    -> Wo_eff_h = Wvr @ Wo_h folded host-side; const row into bo_eff.

Pipeline per chunk c (512 tokens): project q~/latk/latv for chunk c,
emit out-projection for q-block c-1, then attention for q-block c.
Keeps PE dense (no HAM re-throttle) and hides ScalarE exp latency.

Scores for the two heads of a pair are K=64 matmuls running concurrently
on PE row-tiles (0,0)/(64,0). Score tiles go to [128,1024] 2-bank PSUM
groups so one ACTIVATE exps 1024 columns (amortizes the 352-cycle ramp).
Diagonal tiles compute full 512 wide; above-diagonal prefixes are zeroed
by GpSimd memsets, the 128x128 triangle by a mask multiply.

U~ chain lhsT is latv with an appended ones column: out row 64 = softmax
denominator for free. Normalize = replicate row 64 across partitions via
two K=1 M=64 matmuls into one PSUM bank, cast, then tensor_tensor divide.
"""

import math
from contextlib import ExitStack

import numpy as np

import concourse.mybir as mybir
from concourse import bacc
from concourse.bass import ds, ts
from concourse.tile import TileContext

# Problem constants (hardcoded per contract).
B, S, D = 2, 2048, 2048
H, DK, DV, L = 16, 128, 128, 64
N_CORES = 8
HPC = 4                   # heads per core
NPAIR = 2                 # head pairs per core
SB = S                    # tokens per core (its batch)
KO = D // 128             # contraction k-tiles over D = 16
CHUNK = 512               # token chunk for input streaming
NCH = SB // CHUNK         # 4
QT = SB // 128            # 16 token tiles
LW = L + 1                # latv group width (64 latents + ones col)

F32 = mybir.dt.float32
F32R = mybir.dt.float32r
BF16 = mybir.dt.bfloat16

INV_SQRT_DK = 1.0 / math.sqrt(DK)
EXPF = mybir.ActivationFunctionType.Exp


def build_kernel():
    nc = bacc.Bacc(trn_type="TRN2", debug=False, num_swdge_queues=2)

    # ---- DRAM I/O (all host-packed for contiguous DMA) ----
    xq = nc.dram_tensor("xq", [NCH, 128, KO, CHUNK], BF16, kind="ExternalInput")
    xk = nc.dram_tensor("xk", [NCH, 128, KO, CHUNK], BF16, kind="ExternalInput")
    xv = nc.dram_tensor("xv", [NCH, 128, KO, CHUNK], BF16, kind="ExternalInput")
    wq = nc.dram_tensor("wq", [NPAIR, 128, KO, 128], BF16, kind="ExternalInput")
    bq = nc.dram_tensor("bq", [128, NPAIR], F32, kind="ExternalInput")
    wlk = nc.dram_tensor("wlk", [128, KO, 128 * NPAIR], BF16, kind="ExternalInput")
    wlv = nc.dram_tensor("wlv", [128, KO, HPC * L], BF16, kind="ExternalInput")
    wo = nc.dram_tensor("wo", [128, NPAIR, D], BF16, kind="ExternalInput")
    outp = nc.dram_tensor("outp", [QT, 128, D], BF16, kind="ExternalOutput")

    with TileContext(nc) as tc, ExitStack() as ctx:
        ec = ctx.enter_context
        consts = ec(tc.tile_pool(name="consts", bufs=1))
        persist = ec(tc.tile_pool(name="persist", bufs=1))
        xpool = ec(tc.tile_pool(name="xpool", bufs=3))
        ptpool = ec(tc.tile_pool(name="ptpool", bufs=2))
        statpool = ec(tc.tile_pool(name="statpool", bufs=2))
        opool = ec(tc.tile_pool(name="opool", bufs=2))
        psa = ec(tc.tile_pool(name="psa", bufs=2, space="PSUM"))
        pss = ec(tc.tile_pool(name="pss", bufs=2, space="PSUM"))
        psu = ec(tc.tile_pool(name="psu", bufs=2, space="PSUM"))

        # ---- first loads, split small on parallel HWDGE queues so the
        # ---- first q~ chain can start ~13us in instead of ~24us ----
        wq_sb = consts.tile([128, NPAIR, KO, 128], BF16, tag="wq")
        nc.sync.dma_start(wq_sb[:, 0], wq[0])
        x0 = []
        t = xpool.tile([128, KO, CHUNK], BF16, tag="x", name="xq_t")
        nc.scalar.dma_start(t[:, 0:8, :], xq[0][:, 0:8, :])
        nc.scalar.dma_start(t[:, 8:16, :], xq[0][:, 8:16, :])
        x0.append(t)
        nc.sync.dma_start(wq_sb[:, 1], wq[1])
        wlk_sb = consts.tile([128, KO, 128 * NPAIR], BF16, tag="wlk")
        nc.sync.dma_start(wlk_sb, wlk[:, :, :])
        t = xpool.tile([128, KO, CHUNK], BF16, tag="x", name="xk_t")
        nc.scalar.dma_start(t[:, 0:8, :], xk[0][:, 0:8, :])
        nc.scalar.dma_start(t[:, 8:16, :], xk[0][:, 8:16, :])
        x0.append(t)
        wlv_sb = consts.tile([128, KO, HPC * L], BF16, tag="wlv")
        nc.sync.dma_start(wlv_sb, wlv[:, :, :])
        t = xpool.tile([128, KO, CHUNK], BF16, tag="x", name="xv_t")
        nc.scalar.dma_start(t, xv[0])
        x0.append(t)
        bq_sb = consts.tile([128, NPAIR], F32, tag="bq")
        nc.gpsimd.dma_start(bq_sb, bq[:, :])
        wo_sb = consts.tile([128, NPAIR, D], BF16, tag="wo")
        nc.gpsimd.dma_start(wo_sb, wo[:, :, :])

        # causal mask for a diagonal 128x128 block of P~^T: 1 where k <= q
        maskT = consts.tile([128, 128], BF16, tag="maskT")
        nc.gpsimd.memset(maskT, 1.0)
        nc.gpsimd.affine_select(
            out=maskT, in_=maskT, compare_op=mybir.AluOpType.is_ge,
            fill=0.0, base=0, pattern=[[1, 128]], channel_multiplier=-1,
        )
        # all-ones tile: K=1 lhsT rows for denominator replication
        ones_r = consts.tile([128, 128], BF16, tag="ones_r")
        nc.gpsimd.memset(ones_r, 1.0)

        # ---- persistent per-batch tensors ----
        qsb = persist.tile([128, NPAIR, SB], BF16, tag="qsb")
        ksb = persist.tile([128, NPAIR, SB], BF16, tag="ksb")
        vsb = persist.tile([128, QT, HPC * LW], BF16, tag="vsb")
        asb = persist.tile([128, NPAIR, SB], BF16, tag="asb")
        for h in range(HPC):
            nc.gpsimd.memset(vsb[:, :, LW * h + L : LW * h + L + 1], 1.0)

        def norm_unit(Q, usb_q, rcp_q, p, r):
            qsl = ds(Q * 512, 512)
            a_sl = asb[:, p, qsl]
            h = 2 * p + r
            ps_rep = psa.tile([128, 512], F32, tag="s", name="ps_rep")
            nc.tensor.matmul(
                ps_rep, ones_r[32 * h : 32 * h + 1, :],
                rcp_q[32 * h : 32 * h + 1, :],
                start=True, stop=True,
                tile_position=(32 * h, 0),
            )
            nc.vector.tensor_tensor(
                a_sl[64 * r : 64 * r + 64, :],
                usb_q[p][r][0:64, :],
                ps_rep[0:64, :],
                mybir.AluOpType.mult,
            )

        def out_proj_unit(Q, tl):
            tt = Q * 4 + tl
            o_sb = opool.tile([128, D], BF16, tag="o", name="o_sb")
            for dc2 in range(2):
                ps_f = [psa.tile([128, 512], F32, tag="s", name="ps_f")
                        for _ in range(2)]
                for kk in range(NPAIR):
                    for i in range(2):  # one LDWEIGHTS feeds two matmuls
                        nc.tensor.matmul(
                            ps_f[i], asb[:, kk, ts(tt, 128)],
                            wo_sb[:, kk, ts(2 * dc2 + i, 512)],
                            start=(kk == 0), stop=(kk == NPAIR - 1),
                        )
                for i in range(2):
                    nc.any.tensor_copy(out=o_sb[:, ts(2 * dc2 + i, 512)],
                                       in_=ps_f[i])
            nc.sync.dma_start(outp[tt], o_sb)

        for c in range(NCH):
            # ---- stream chunk c and project q~ / latk / latv ----
            if c == 0:
                xq_t, xk_t, xv_t = x0
            else:
                xq_t = xpool.tile([128, KO, CHUNK], BF16, tag="x")
                nc.sync.dma_start(xq_t, xq[c])
                xk_t = xpool.tile([128, KO, CHUNK], BF16, tag="x")
                nc.sync.dma_start(xk_t, xk[c])
                xv_t = xpool.tile([128, KO, CHUNK], BF16, tag="x")
                nc.scalar.dma_start(xv_t, xv[c])

            csl = ds(c * CHUNK, CHUNK)
            for m in range(NPAIR):
                ps = psa.tile([128, 512], F32, tag="s", name="ps_q")
                for ko in range(KO):
                    nc.tensor.matmul(
                        ps, wq_sb[:, m, ko, :], xq_t[:, ko, :],
                        start=(ko == 0), stop=(ko == KO - 1),
                    )
                nc.vector.tensor_scalar_add(
                    qsb[:, m, csl], ps, bq_sb[:, m : m + 1])
            for m in range(NPAIR):
                ps = psa.tile([128, 512], F32, tag="s", name="ps_k")
                for ko in range(KO):
                    nc.tensor.matmul(
                        ps, wlk_sb[:, ko, ts(m, 128)], xk_t[:, ko, :],
                        start=(ko == 0), stop=(ko == KO - 1),
                    )
                nc.any.tensor_copy(out=ksb[:, m, csl], in_=ps)
            # latv: token-major, [128 tok, 256] per token tile
            for tl in range(4):
                tt = c * 4 + tl
                ps = psa.tile([128, 512], F32, tag="s", name="ps_v")
                for ko in range(KO):
                    nc.tensor.matmul(
                        ps[:, : HPC * L], xv_t[:, ko, ts(tl, 128)],
                        wlv_sb[:, ko, :],
                        start=(ko == 0), stop=(ko == KO - 1),
                    )
                for h in range(HPC):
                    nc.any.tensor_copy(
                        out=vsb[:, tt, ds(LW * h, L)],
                        in_=ps[:, ds(L * h, L)],
                    )

            # ---- attention for q-block Q = c, woven with the previous
            # ---- q-block's normalize + out-projection as PE filler ----
            Q = c
            jmax = 4 * Q + 4
            qsl = ds(Q * 512, 512)

            filler = []
            if c > 0:
                pv, uq, rq = c - 1, usb_q, rcp_q
                for pp in range(NPAIR):
                    for rr in range(2):
                        filler.append(
                            (lambda pp=pp, rr=rr: norm_unit(pv, uq, rq,
                                                            pp, rr)))
                for tl in range(4):
                    filler.append(lambda tl=tl, pv=pv: out_proj_unit(pv, tl))

            usb_q = [None] * NPAIR
            den = statpool.tile([128, 512], F32, tag="den", name="den")
            rcp_q = statpool.tile([128, 512], BF16, tag="rcp", name="rcp")

            ngroups = NPAIR * (jmax // 2)
            gdone = 0
            fdone = 0
            for p in range(NPAIR):
                pt = [ptpool.tile([128, QT, 512], BF16, tag=f"pt{r}",
                                  name=f"pt{r}") for r in range(2)]
                ps_u = [psu.tile([65, 512], F32, tag="u", name="ps_u")
                        for _ in range(2)]

                def u_group(g):
                    for j2 in range(2):
                        j = 2 * g + j2
                        for r in range(2):
                            nc.tensor.matmul(
                                ps_u[r],
                                vsb[:, j, ds(LW * (2 * p + r), LW)],
                                pt[r][:, j, :],
                                start=(j == 0), stop=(j == jmax - 1),
                            )

                for g in range(jmax // 2):
                    # scores: r-adjacent so the two heads run on PE
                    # row-tiles (0,0)/(64,0) concurrently
                    sgs = [pss.tile([128, 1024], F32, tag="sg", name="sg")
                           for _ in range(2)]
                    for j2 in range(2):
                        j = 2 * g + j2
                        for r in range(2):
                            rs = slice(64 * r, 64 * r + 64)
                            nc.tensor.matmul(
                                sgs[r][:, ds(512 * j2, 512)],
                                ksb[rs, p, ts(j, 128)], qsb[rs, p, qsl],
                                start=True, stop=True,
                            )
                    for r in range(2):
                        nc.scalar.activation(
                            pt[r][:, 2 * g : 2 * g + 2, :], sgs[r],
                            EXPF, scale=INV_SQRT_DK,
                        )
                    # causal masking on diagonal k-tiles (GpSimd: idle)
                    for j2 in range(2):
                        j = 2 * g + j2
                        i = j - 4 * Q
                        if i < 0:
                            continue
                        for r in range(2):
                            if i > 0:
                                nc.gpsimd.memset(
                                    pt[r][:, j, ds(0, 128 * i)], 0.0)
                            nc.gpsimd.tensor_tensor(
                                pt[r][:, j, ds(128 * i, 128)],
                                pt[r][:, j, ds(128 * i, 128)],
                                maskT, mybir.AluOpType.mult,
                            )
                    # U~ for the previous group (exp had time to finish)
                    if g > 0:
                        u_group(g - 1)
                    gdone += 1
                    # weave in filler units proportionally
                    want = (gdone * len(filler)) // ngroups
                    while fdone < want:
                        filler[fdone]()
                        fdone += 1
                u_group(jmax // 2 - 1)

                # evacuate U~ to SBUF (frees PSUM); pack the q-block's 4
                # denominator rows at 32-aligned partitions of one tile
                usb_q[p] = [statpool.tile([128, 512], BF16,
                                          tag=f"usb{2 * p + r}", name="usb")
                            for r in range(2)]
                for r in range(2):
                    h = 2 * p + r
                    nc.any.tensor_copy(out=usb_q[p][r][0:64, :],
                                       in_=ps_u[r][0:64, :])
                    nc.any.tensor_copy(
                        out=den[32 * h : 32 * h + 1, :],
                        in_=ps_u[r][64:65, :])

            while fdone < len(filler):
                filler[fdone]()
                fdone += 1

            # one reciprocal covers all 4 heads of this q-block
            with nc.allow_low_precision(reason="1/d in bf16 adds ~0.4% "
                                        "scale noise, within tolerance"):
                nc.vector.reciprocal(rcp_q, den)

        for pp in range(NPAIR):
            for rr in range(2):
                norm_unit(NCH - 1, usb_q, rcp_q, pp, rr)
        for tl in range(4):
            out_proj_unit(NCH - 1, tl)

    nc.finalize()
    return nc


_NC_CACHE = None


def _get_nc():
    global _NC_CACHE
    if _NC_CACHE is None:
        _NC_CACHE = build_kernel()
    return _NC_CACHE


def _pack_xT(Xb, bf16):
    # Xb [S, D] fp32 -> X^T packed [NCH, 128, KO, CHUNK] (d = ko*128 + p)
    xt = np.asarray(Xb).T.reshape(KO, 128, NCH, CHUNK)
    return np.ascontiguousarray(xt.transpose(2, 1, 0, 3).astype(bf16))


def _prep_in_maps(queries, keys, values, Wq, bq, Wlk, blk, Wlv, blv,
                  Wkr, bkr, Wvr, bvr, Wo, bo):
    import ml_dtypes

    bf16 = ml_dtypes.bfloat16
    f = np.float32
    Wq, bq, Wlk, Wlv = (np.asarray(a, f) for a in (Wq, bq, Wlk, Wlv))
    Wkr, Wvr, Wo = (np.asarray(a, f) for a in (Wkr, Wvr, Wo))

    # host-side absorption folds (exact algebra)
    # Wq_eff_h = Wq_h @ Wkr^T [D, L]; bq_eff_h = bq_h @ Wkr^T
    WqH = Wq.reshape(D, H, DK)
    Wq_eff = np.einsum("dhk,lk->dhl", WqH, Wkr).reshape(D, H * L)
    bq_eff = (bq.reshape(H, DK) @ Wkr.T).reshape(H * L)
    # Wo_eff_h = Wvr @ Wo_h [L, D]
    WoH = Wo.reshape(H, DV, D)
    Wo_eff = np.einsum("lk,hkd->hld", Wvr, WoH).reshape(H * L, D)

    in_maps = []
    for c in range(N_CORES):
        b, hg = c // 4, c % 4
        hsl = slice(hg * 4 * L, (hg + 1) * 4 * L)     # 4 heads' latent cols

        xq_c = _pack_xT(queries[b], bf16)
        xk_c = _pack_xT(keys[b], bf16)
        xv_c = _pack_xT(values[b], bf16)

        wq_c = np.ascontiguousarray(
            Wq_eff[:, hsl].reshape(KO, 128, NPAIR, 128)
            .transpose(2, 1, 0, 3).astype(bf16))
        bq_c = np.ascontiguousarray(
            bq_eff[hsl].reshape(NPAIR, 128).T, f)
        wlk_c = np.ascontiguousarray(
            Wlk[:, hsl].reshape(KO, 128, 256).transpose(1, 0, 2).astype(bf16))
        wlv_c = np.ascontiguousarray(
            Wlv[:, hsl].reshape(KO, 128, 256).transpose(1, 0, 2).astype(bf16))
        wo_c = np.ascontiguousarray(
            Wo_eff[hsl, :].reshape(NPAIR, 128, D).transpose(1, 0, 2)
            .astype(bf16))

        in_maps.append({
            "xq": xq_c, "xk": xk_c, "xv": xv_c,
            "wq": wq_c, "bq": bq_c, "wlk": wlk_c, "wlv": wlv_c, "wo": wo_c,
        })
    return in_maps


def _assemble(results, inputs):
    f64 = np.float64
    blv = np.asarray(inputs["blv"], f64).reshape(H, L)
    bvr = np.asarray(inputs["bvr"], f64)
    Wvr = np.asarray(inputs["Wvr"], f64)
    WoH = np.asarray(inputs["Wo"], f64).reshape(H, DV, D)
    bo_eff = np.asarray(inputs["bo"], f64).copy()
    for h in range(H):
        bo_eff += (blv[h] @ Wvr + bvr) @ WoH[h]

    out = np.zeros((B, S, D), f64)
    for c, rmap in enumerate(results):
        out[c // 4] += rmap["outp"].reshape(S, D).astype(f64)
    out += bo_eff
    return out.astype(np.float32)


def kernel(**inputs):
    from concourse.bass_utils import run_bass_kernel_spmd

    nc = _get_nc()
    in_maps = _prep_in_maps(**inputs)
    res = run_bass_kernel_spmd(
        nc, in_maps, core_ids=list(range(N_CORES)), trace=False
    )
    return _assemble(res.results, inputs)


if __name__ == "__main__":
    nc = build_kernel()
    print("built ok, instructions:", len(nc.inst_map))
